# revision 14
# baseline (speedup 1.0000x reference)
"""Trainium2 Bass kernel: depthwise (per-sample, per-channel) 15x15 'same'
true convolution of 1024x3 images of 128x128, data-parallel over 8 NeuronCores.

Formulation (per (bn,c) pair, P=128, K=15, pad=7):
    out[y,x] = sum_{dy,dx} Xp[y+dy, x+dx] * Wf[dy,dx],   Wf = flip(kernel),
    Xp = zero-padded image [142, 143].
One matmul per dx pass (15 total per pair): stationary Toeplitz slab
T[i, j] = Wf[i-j, dx] ([46, 32]); the moving operand spans all four 32-row
output blocks at once ([46, (b,x)] = 512 columns, block b's window = image
rows 32b..32b+45, column offset dx), producing out[j, (b, x)] in PSUM
[32, 512] — exactly one bank. This keeps the PE sequencer instruction count
(the measured ~16.6 ns/instruction dispatch floor) at 30 per pair instead of
120. Passes ping-pong between two array tiles — even dx on tile (0,0) (rows
0..45, PSUM partitions 0..31), odd dx on tile (64,32) (rows 64..109, PSUM
partitions 32..63) — so each tile's next LDWEIGHTS targets disjoint row
groups and pulls ahead of the other tile's in-flight matmul. The two PSUM
accumulators are evacuated partition-aligned (DVE rows 0..31, ACT rows
32..63) into a [64, 512]-per-pair fp16 output; the host sums the two
parities and reassembles [y, x].

Data staging (per group of G=32 pairs, all via gpsimd SWDGE, which stripes
descriptors across DMA engines 2..15 — the two HWDGE rings are pinned to
engines 0/1): images stored pair-interleaved in DRAM ([row][pair][143]); the
block-window tile ([128, 4*G*143], windows at partitions 0..45, duplicated at
64..109 by an SBUF->SBUF DMA) loads with 9152-byte runs. Sharding: pure data
parallel over BN (128 samples x 3 channels = 384 pairs per core).
"""
import sys

sys.path.insert(0, "/opt/trn_rl_repo")

import numpy as np

_N_CORES = 8
_BN, _C, _P, _K = 1024, 3, 128, 15
_PAIRS_PER_CORE = (_BN // _N_CORES) * _C  # 384
_G = 32                      # pairs per DMA group
_NG = _PAIRS_PER_CORE // _G  # 12
_XW = 143                    # padded image width (cols 0..142)
_XH = 142                    # padded image height
_ROWP = _G * _XW             # elems per padded row across a group (4576)
_GRP = _XH * _ROWP           # elems per group image block
_XCW = 4 * _ROWP             # XC tile free width (18304)
_TSLAB = _G * 8 * 32         # T free elems per partition per group

_nc_cache = {}


def _build_nc(bufs: int = 2, psum_bufs: int = 4):
    import concourse.bacc as bacc
    import concourse.mybir as mybir
    from concourse import bass, tile

    FP16 = mybir.dt.float16
    FP32 = mybir.dt.float32

    nc = bacc.Bacc("TRN2", target_bir_lowering=False, debug=False)
    xpad_d = nc.dram_tensor("xpad", [_NG * _GRP + 64], FP16, kind="ExternalInput")
    toep_d = nc.dram_tensor("toep", [_NG, 2, 46, _TSLAB], FP16, kind="ExternalInput")
    out_d = nc.dram_tensor("out", [_NG, 64, _G * 512], FP16, kind="ExternalOutput")
    xt = xpad_d.tensor if hasattr(xpad_d, "tensor") else xpad_d

    with tile.TileContext(nc) as tc:
        with (
            tc.tile_pool(name="xc", bufs=bufs) as xc_pool,
            tc.tile_pool(name="tt", bufs=bufs) as tt_pool,
            tc.tile_pool(name="ot", bufs=bufs) as ot_pool,
            tc.tile_pool(name="ps", bufs=psum_bufs, space="PSUM") as ps_pool,
        ):
            for grp in range(_NG):
                xc = xc_pool.tile([128, _XCW], FP16, tag="xc")
                tt = tt_pool.tile([128, _TSLAB], FP16, tag="tt")
                ot = ot_pool.tile([64, _G * 512], FP16, tag="ot")

                # windows rows 32b..32b+45 (block-major, pair-interleaved)
                src0 = bass.AP(
                    tensor=xt,
                    offset=grp * _GRP,
                    ap=[[_ROWP, 46], [32 * _ROWP, 4], [_XW, _G], [1, _XW]],
                )
                nc.gpsimd.dma_start(out=xc[0:46, 0:_XCW], in_=src0)
                # duplicate at partitions 64..109 for the odd-parity tile
                nc.gpsimd.dma_start(out=xc[64:110, 0:_XCW], in_=xc[0:46, 0:_XCW])
                nc.gpsimd.dma_start(out=tt[0:46, :], in_=toep_d[grp, 0])
                nc.gpsimd.dma_start(out=tt[64:110, :], in_=toep_d[grp, 1])

                xcap = xc[:]
                for g in range(_G):
                    psA = ps_pool.tile([128, 512], FP32, tag="psA")
                    psB = ps_pool.tile([128, 512], FP32, tag="psB")
                    ps = (psA, psB)
                    for dx in range(15):
                        p = dx & 1
                        slot = dx >> 1
                        rhs = bass.AP(
                            tensor=xcap.tensor,
                            offset=xcap.offset + 64 * p * _XCW
                            + g * _XW + dx,
                            ap=[[_XCW, 46], [_ROWP, 4], [1, 128]],
                        )
                        nc.tensor.matmul(
                            ps[p][32 * p:32 * p + 32, :],
                            tt[64 * p:64 * p + 46,
                               (g * 8 + slot) * 32:(g * 8 + slot) * 32 + 32],
                            rhs,
                            start=(dx < 2), stop=(dx >= 13),
                            tile_position=(64 * p, 32 * p),
                        )
                    nc.vector.tensor_copy(
                        ot[0:32, g * 512:(g + 1) * 512], psA[0:32, :])
                    nc.scalar.copy(
                        ot[32:64, g * 512:(g + 1) * 512], psB[32:64, :])

                nc.gpsimd.dma_start(out=out_d[grp], in_=ot[:])

    nc.compile()
    return nc


def _host_prep(patches_pairs: np.ndarray, kernels_pairs: np.ndarray):
    """[NP,128,128] f32, [NP,15,15] f32 -> (xpad flat fp16, toep fp16).

    xpad: [NG*142*G*143 + 64] with layout [grp][row 142][pair G][col 143],
    zero-padded images at rows/cols 7..134.
    toep: [NG, 2, 46, G, 8, 32]: parity 0 slots hold dx=2e, parity 1 slots
    dx=2o+1, T[i, slot, j] = Wf[i-j, dx] for 0 <= i-j < 15.
    """
    NP = patches_pairs.shape[0]
    assert NP == _PAIRS_PER_CORE
    Xp = np.zeros((_NG, _G, _XH, _XW), dtype=np.float16)
    Xp[:, :, 7:135, 7:135] = patches_pairs.reshape(_NG, _G, 128, 128)
    xpad = np.zeros(_NG * _GRP + 64, dtype=np.float16)
    xpad[:_NG * _GRP] = np.ascontiguousarray(
        Xp.transpose(0, 2, 1, 3)).reshape(-1)

    Wf = np.ascontiguousarray(
        kernels_pairs[:, ::-1, ::-1]).astype(np.float16)  # [NP, 15, 15]
    T = np.zeros((NP, 2, 46, 8, 32), dtype=np.float16)
    j = np.arange(32)
    for dy in range(15):
        for slot in range(8):
            for par in range(2):
                dx = 2 * slot + par
                if dx > 14:
                    continue
                T[:, par, j + dy, slot, j] = Wf[:, dy, dx][:, None]
    T = T.reshape(_NG, _G, 2, 46, 8 * 32).transpose(0, 2, 3, 1, 4)
    toep = np.ascontiguousarray(T).reshape(_NG, 2, 46, _TSLAB)
    return xpad, toep


def kernel(patches, kernels, kernel_size, patch_size, fft_size, _collect_results=None):
    """Full inputs in, full output out. Shards BN across 8 cores."""
    from concourse.bass_utils import run_bass_kernel_spmd

    patches = np.asarray(patches)
    kernels = np.asarray(kernels)
    assert patches.shape == (_BN, _C, _P, _P), patches.shape
    assert kernels.shape == (_BN, _C, _K, _K), kernels.shape

    if "nc" not in _nc_cache:
        _nc_cache["nc"] = _build_nc()
    nc = _nc_cache["nc"]

    bn_per_core = _BN // _N_CORES
    in_maps = []
    for core in range(_N_CORES):
        sl = slice(core * bn_per_core, (core + 1) * bn_per_core)
        pp = patches[sl].reshape(-1, _P, _P)
        kp = kernels[sl].reshape(-1, _K, _K)
        xpad, toep = _host_prep(pp, kp)
        in_maps.append({"xpad": xpad, "toep": toep})

    res = run_bass_kernel_spmd(nc, in_maps, core_ids=list(range(_N_CORES)))
    if _collect_results is not None:
        _collect_results.append(res)

    out = np.empty((_BN, _C, _P, _P), dtype=np.float32)
    for core in range(_N_CORES):
        sl = slice(core * bn_per_core, (core + 1) * bn_per_core)
        o = res.results[core]["out"].reshape(_NG, 2, 32, _G, 4, 128)
        s = o[:, 0].astype(np.float32) + o[:, 1].astype(np.float32)
        # [NG, 32j, G, 4b, 128x] -> [NG, G, 4b, 32j, 128x] -> [pairs, y, x]
        out[sl] = s.transpose(0, 2, 3, 1, 4).reshape(
            bn_per_core, _C, _P, _P)
    return out


# revision 15
# speedup vs baseline: 1.5428x; 1.5428x over previous
"""Trainium2 Bass kernel: depthwise (per-sample, per-channel) 15x15 'same'
true convolution of 1024x3 images of 128x128, data-parallel over 8 NeuronCores.

Formulation (per (bn,c) pair, P=128, K=15, pad=7):
    out[y,x] = sum_{dy,dx} Xp[y+dy, x+dx] * Wf[dy,dx],   Wf = flip(kernel),
    Xp = zero-padded image [142, 143].
One matmul per dx pass (15 total per pair): stationary Toeplitz slab
T[i, j] = Wf[i-j, dx] ([46, 32]); the moving operand spans all four 32-row
output blocks at once ([46, (b,x)] = 512 columns, block b's window = image
rows 32b..32b+45, column offset dx), producing out[j, (b, x)] in PSUM
[32, 512] — exactly one bank. This keeps the PE sequencer instruction count
(the measured ~16.6 ns/instruction dispatch floor) at 30 per pair instead of
120. Passes ping-pong between two array tiles — even dx on tile (0,0) (rows
0..45, PSUM partitions 0..31), odd dx on tile (64,32) (rows 64..109, PSUM
partitions 32..63) — so each tile's next LDWEIGHTS targets disjoint row
groups and pulls ahead of the other tile's in-flight matmul. The two PSUM
accumulators are evacuated partition-aligned (DVE rows 0..31, ACT rows
32..63) into a [64, 512]-per-pair fp16 output; the host sums the two
parities and reassembles [y, x].

Data staging (per group of G=32 pairs, all via gpsimd SWDGE, which stripes
descriptors across DMA engines 2..15 — the two HWDGE rings are pinned to
engines 0/1): images stored pair-interleaved in DRAM ([row][pair][143]); the
block-window tile ([128, 4*G*143], windows at partitions 0..45, duplicated at
64..109 by an SBUF->SBUF DMA) loads with 9152-byte runs. Sharding: pure data
parallel over BN (128 samples x 3 channels = 384 pairs per core).
"""
import sys

sys.path.insert(0, "/opt/trn_rl_repo")

import numpy as np

_N_CORES = 8
_BN, _C, _P, _K = 1024, 3, 128, 15
_PAIRS_PER_CORE = (_BN // _N_CORES) * _C  # 384
_G = 32                      # pairs per DMA group
_NG = _PAIRS_PER_CORE // _G  # 12
_XW = 143                    # padded image width (cols 0..142)
_XH = 142                    # padded image height
_ROWP = _G * _XW             # elems per padded row across a group (4576)
_GRP = _XH * _ROWP           # elems per group image block
_XCW = 4 * _ROWP             # XC tile free width (18304)
_TSLAB = _G * 8 * 32         # T free elems per partition per group

_nc_cache = {}


def _build_nc(bufs: int = 2, psum_bufs: int = 4):
    import concourse.bacc as bacc
    import concourse.mybir as mybir
    from concourse import bass, tile

    FP16 = mybir.dt.float16
    FP32 = mybir.dt.float32

    nc = bacc.Bacc("TRN2", target_bir_lowering=False, debug=False)
    xpad_d = nc.dram_tensor("xpad", [_NG * _GRP + 64], FP16, kind="ExternalInput")
    toep_d = nc.dram_tensor("toep", [_NG, 2, 46, _TSLAB], FP16, kind="ExternalInput")
    out_d = nc.dram_tensor("out", [_NG, 128, _G * 256], FP16, kind="ExternalOutput")
    xt = xpad_d.tensor if hasattr(xpad_d, "tensor") else xpad_d

    with tile.TileContext(nc) as tc:
        with (
            tc.tile_pool(name="xc", bufs=bufs) as xc_pool,
            tc.tile_pool(name="tt", bufs=bufs) as tt_pool,
            tc.tile_pool(name="ot", bufs=bufs) as ot_pool,
            tc.tile_pool(name="ps", bufs=psum_bufs, space="PSUM") as ps_pool,
        ):
            for grp in range(_NG):
                xc = xc_pool.tile([128, _XCW], FP16, tag="xc")
                tt = tt_pool.tile([128, _TSLAB], FP16, tag="tt")
                ot = ot_pool.tile([128, _G * 256], FP16, tag="ot")

                # windows rows 32b..32b+45 (block-major, pair-interleaved)
                src0 = bass.AP(
                    tensor=xt,
                    offset=grp * _GRP,
                    ap=[[_ROWP, 46], [32 * _ROWP, 4], [_XW, _G], [1, _XW]],
                )
                nc.gpsimd.dma_start(out=xc[0:46, 0:_XCW], in_=src0)
                # duplicate at partitions 64..109 for the odd-parity tile
                nc.gpsimd.dma_start(out=xc[64:110, 0:_XCW], in_=xc[0:46, 0:_XCW])
                nc.gpsimd.dma_start(out=tt[0:46, :], in_=toep_d[grp, 0])
                nc.gpsimd.dma_start(out=tt[64:110, :], in_=toep_d[grp, 1])

                xcap = xc[:]
                for g in range(_G):
                    psE = ps_pool.tile([128, 512], FP32, tag="psE")
                    psO = ps_pool.tile([128, 512], FP32, tag="psO")
                    for dx in range(15):
                        p = dx & 1
                        slot = dx >> 1
                        for h in range(2):
                            s = 2 * p + h
                            rhs = bass.AP(
                                tensor=xcap.tensor,
                                offset=xcap.offset + 64 * p * _XCW
                                + 2 * h * _ROWP + g * _XW + dx,
                                ap=[[_XCW, 46], [_ROWP, 2], [1, 128]],
                            )
                            out_ps = (psE[32 * h:32 * h + 32, 0:256] if p == 0
                                      else psO[64 + 32 * h:96 + 32 * h, 0:256])
                            nc.tensor.matmul(
                                out_ps,
                                tt[64 * p:64 * p + 46,
                                   (g * 8 + slot) * 32:(g * 8 + slot) * 32 + 32],
                                rhs,
                                start=(dx < 2), stop=(dx >= 13),
                                tile_position=(64 * p, 32 * s),
                            )
                    nc.vector.tensor_copy(
                        ot[0:64, g * 256:(g + 1) * 256], psE[0:64, 0:256])
                    nc.scalar.copy(
                        ot[64:128, g * 256:(g + 1) * 256], psO[64:128, 0:256])

                nc.gpsimd.dma_start(out=out_d[grp], in_=ot[:])

    nc.compile()
    return nc


def _host_prep(patches_pairs: np.ndarray, kernels_pairs: np.ndarray):
    """[NP,128,128] f32, [NP,15,15] f32 -> (xpad flat fp16, toep fp16).

    xpad: [NG*142*G*143 + 64] with layout [grp][row 142][pair G][col 143],
    zero-padded images at rows/cols 7..134.
    toep: [NG, 2, 46, G, 8, 32]: parity 0 slots hold dx=2e, parity 1 slots
    dx=2o+1, T[i, slot, j] = Wf[i-j, dx] for 0 <= i-j < 15.
    """
    NP = patches_pairs.shape[0]
    assert NP == _PAIRS_PER_CORE
    Xp = np.zeros((_NG, _G, _XH, _XW), dtype=np.float16)
    Xp[:, :, 7:135, 7:135] = patches_pairs.reshape(_NG, _G, 128, 128)
    xpad = np.zeros(_NG * _GRP + 64, dtype=np.float16)
    xpad[:_NG * _GRP] = np.ascontiguousarray(
        Xp.transpose(0, 2, 1, 3)).reshape(-1)

    Wf = np.ascontiguousarray(
        kernels_pairs[:, ::-1, ::-1]).astype(np.float16)  # [NP, 15, 15]
    T = np.zeros((NP, 2, 46, 8, 32), dtype=np.float16)
    j = np.arange(32)
    for dy in range(15):
        for slot in range(8):
            for par in range(2):
                dx = 2 * slot + par
                if dx > 14:
                    continue
                T[:, par, j + dy, slot, j] = Wf[:, dy, dx][:, None]
    T = T.reshape(_NG, _G, 2, 46, 8 * 32).transpose(0, 2, 3, 1, 4)
    toep = np.ascontiguousarray(T).reshape(_NG, 2, 46, _TSLAB)
    return xpad, toep


def kernel(patches, kernels, kernel_size, patch_size, fft_size, _collect_results=None):
    """Full inputs in, full output out. Shards BN across 8 cores."""
    from concourse.bass_utils import run_bass_kernel_spmd

    patches = np.asarray(patches)
    kernels = np.asarray(kernels)
    assert patches.shape == (_BN, _C, _P, _P), patches.shape
    assert kernels.shape == (_BN, _C, _K, _K), kernels.shape

    if "nc" not in _nc_cache:
        _nc_cache["nc"] = _build_nc()
    nc = _nc_cache["nc"]

    bn_per_core = _BN // _N_CORES
    in_maps = []
    for core in range(_N_CORES):
        sl = slice(core * bn_per_core, (core + 1) * bn_per_core)
        pp = patches[sl].reshape(-1, _P, _P)
        kp = kernels[sl].reshape(-1, _K, _K)
        xpad, toep = _host_prep(pp, kp)
        in_maps.append({"xpad": xpad, "toep": toep})

    res = run_bass_kernel_spmd(nc, in_maps, core_ids=list(range(_N_CORES)))
    if _collect_results is not None:
        _collect_results.append(res)

    out = np.empty((_BN, _C, _P, _P), dtype=np.float32)
    for core in range(_N_CORES):
        sl = slice(core * bn_per_core, (core + 1) * bn_per_core)
        o = res.results[core]["out"].reshape(_NG, 2, 2, 32, _G, 2, 128)
        s = o[:, 0].astype(np.float32) + o[:, 1].astype(np.float32)
        # [NG, 2h, 32j, G, 2db, 128x] -> [NG, G, h, db, j, x] -> [pairs, y, x]
        out[sl] = s.transpose(0, 3, 1, 4, 2, 5).reshape(
            bn_per_core, _C, _P, _P)
    return out
